# revision 24
# baseline (speedup 1.0000x reference)
"""Fused multi-head attention with Transformer-XL relative position bias.

8-way head-parallel Bass/Tile kernel for TRN2 (one core per head), optimized
for end-to-end wall time over the axon tunnel: the dominant cost is host->
device upload bandwidth (~50 MB/s), so inputs are shipped sharded/packed in
fp16 (~11 MB/call vs ~105 MB for the naive duplicated layout) and
reassembled on device with one AllGather.

Math trick (same as baseline): the relative-position band term
band[q,k] = q_q . emb_{q-k} is a matmul because
sin(w(q-k)+p) = sin(wq+p)cos(wk) - cos(wq+p)sin(wk).  With
t = q @ positional^T ([q,64]), u = [t*sinQ, -t*cosQ] ([q,128]) and
c = [cosK, sinK] ([k,128]), band = u @ c^T exactly.  Logits are computed
directly in transposed [k, q] layout; softmax denominators come from a
ones-column in the AV matmul.  Softmax is shift-invariant, so exp uses a
constant -4 bias to keep probabilities in fp16 range (logits are in
[-10, 10] for this input distribution).

Per-call data flow:
  host: xT = x^T (fp16), sharded by sequence; per-head weights packed into
        one fp16 consts array per core; sinusoid tables are inline NEFF
        constants (no upload).
  core c: AllGather xT shards -> full xT; project q/k/v + band inputs for
        head c; attention (fp16 matmuls, fp32 PSUM); row-parallel output
        Dense partial (out_w rows of head c); ReduceScatter(add) -> core c
        owns output sequence slice [256c:256c+256] -> fp16 output.
  host: assemble 8 slices, transpose to [B, S, X] fp32.
"""

import numpy as np

B, S, X = 2, 2048, 512
HEADS, HD = 8, 64
FREQS, MAX_PERIOD = 64, 10000
N_CORES = 8
QS = S // N_CORES  # 256: per-core output sequence slice

NQT = S // 128   # 16 k tiles of 128
NQC = S // 512   # 4 q chunks of 512
NDT = X // 128   # 4 contraction tiles of 128

# consts packing (fp16 [128, CC]) column offsets
C_WQK = 0            # [128, 4*128]  wqk packed (dt, x%128, qk-col)
C_WV = 512           # [128, 4*64]   wv packed
C_POS = 768          # [64 rows, 64] pos[h, f]
C_QB = 832           # [64 rows, 1]  q bias
C_OW = 833           # [64 rows, 512] out_w rows of this head
C_OB = 1345          # [128, 4]      out_b
CC = 1349

XT_SZ = B * NDT * 128 * QS   # 524288 fp16 elems: this core's xT seq-shard
CON_SZ = 128 * CC            # 172672 fp16 elems: packed per-head consts
TOT_IN = XT_SZ + CON_SZ

_CACHE = {}


def _host_constants():
    idx = np.arange(FREQS)
    freq = np.pi * (2 / MAX_PERIOD) ** (idx // 2 / (FREQS // 2 - 1))
    phase = np.pi / 2 * (idx % 2)
    t = np.arange(S)
    arg_q = freq[None, :] * t[:, None] + phase[None, :]  # [q, f]
    csq = np.concatenate([np.sin(arg_q), -np.cos(arg_q)], axis=1).T  # [128, S]
    arg_k = freq[None, :] * t[:, None]  # [k, f]
    csk = np.concatenate([np.cos(arg_k), np.sin(arg_k)], axis=1).T  # [128, S]
    kl = np.arange(128)[:, None]
    jl = np.arange(128)[None, :]
    maskadd = np.where(jl >= kl, 0.0, -1e5)  # [128 k, 128 q]
    return (csq.astype(np.float32), csk.astype(np.float16),
            maskadd.astype(np.float32))


def _build():
    import concourse.mybir as mybir
    from concourse import bacc
    from concourse.tile import TileContext

    f32 = mybir.dt.float32
    f16 = mybir.dt.float16

    csq_h, csk_h, mask_h = _host_constants()

    nc = bacc.Bacc(num_devices=N_CORES, trn_type="TRN2")

    agin = nc.declare_dram_parameter("agin", [TOT_IN], f16, isOutput=False)
    out = nc.declare_dram_parameter("out", [B, X, QS], f16, isOutput=True)

    csq_d = nc.inline_tensor(csq_h, name="csq")
    csk_d = nc.inline_tensor(csk_h, name="csk")
    mask_d = nc.inline_tensor(mask_h, name="maskadd")

    ag_int = nc.dram_tensor("ag_int", [XT_SZ], f16)
    ag_out = nc.dram_tensor("ag_out", [N_CORES, XT_SZ], f16,
                            addr_space="Shared")
    rs_in = nc.dram_tensor("rs_in", [N_CORES, B, X, QS], f32)
    rs_swap = nc.dram_tensor("rs_swap", [N_CORES, B, X, QS], f32)

    groups = [list(range(N_CORES))]

    with TileContext(nc) as tc:
        with tc.tile_pool(name="const", bufs=1) as cpool, \
             tc.tile_pool(name="xt", bufs=2) as xtpool, \
             tc.tile_pool(name="kq", bufs=2) as kqpool, \
             tc.tile_pool(name="vv", bufs=2) as vpool, \
             tc.tile_pool(name="pt", bufs=2) as ptpool, \
             tc.tile_pool(name="sm", bufs=2) as smpool, \
             tc.tile_pool(name="po", bufs=3) as popool, \
             tc.tile_pool(name="fo", bufs=4) as fopool, \
             tc.tile_pool(name="ps512", bufs=4, space="PSUM") as ps512, \
             tc.tile_pool(name="pso", bufs=2, space="PSUM") as pso:

            # Collectives cannot read IO tensors: stage the input shard into
            # internal DRAM first, then AllGather (overlaps const loads).
            nc.sync.dma_start(out=ag_int[:], in_=agin[0:XT_SZ])
            tc.strict_bb_all_engine_barrier()
            nc.gpsimd.collective_compute(
                "AllGather", mybir.AluOpType.bypass,
                replica_groups=groups,
                ins=[ag_int[:]], outs=[ag_out[:]])

            # ---- constants to SBUF ----
            csq_sb = cpool.tile([128, S], f32)
            nc.sync.dma_start(out=csq_sb[:], in_=csq_d[:])
            csk_sb = cpool.tile([128, S], f16)
            nc.sync.dma_start(out=csk_sb[:], in_=csk_d[:])
            mask_sb = cpool.tile([128, 128], f32)
            nc.sync.dma_start(out=mask_sb[:], in_=mask_d[:])
            con_sb = cpool.tile([128, CC], f16)
            nc.sync.dma_start(out=con_sb[:],
                              in_=agin[XT_SZ:TOT_IN].rearrange("(p c) -> p c", p=128))

            tc.strict_bb_all_engine_barrier()

            # fp32 copies of the tiny bias vectors (activation bias wants f32)
            qb32 = cpool.tile([HD, 1], f32)
            nc.vector.tensor_copy(qb32[:], con_sb[0:HD, C_QB:C_QB + 1])
            ob32 = cpool.tile([128, NDT], f32)
            nc.vector.tensor_copy(ob32[:], con_sb[:, C_OB:C_OB + NDT])
            nb4 = cpool.tile([128, 1], f32)
            nc.vector.memset(nb4[:], -4.0)

            for b in range(B):
                # ---- xT from the AllGather: [128, NDT, S] fp16 ----
                xt_sb = xtpool.tile([128, NDT, S], f16, tag="xt", name=f"xt_{b}")
                bsz = NDT * 128 * QS
                for g in range(N_CORES):
                    nc.sync.dma_start(
                        out=xt_sb[:, :, QS * g:QS * g + QS],
                        in_=ag_out[g, b * bsz:(b + 1) * bsz].rearrange(
                            "(d p s) -> p d s", d=NDT, p=128))

                # ---- projections ----
                qT_sb = kqpool.tile([HD, S], f16, tag="qT", name=f"qT_{b}")
                kT_sb = kqpool.tile([HD, S], f16, tag="kT", name=f"kT_{b}")
                for ch in range(NQC):
                    sl = slice(512 * ch, 512 * ch + 512)
                    ps = ps512.tile([128, 512], f32, tag='ps', bufs=2)
                    for dt in range(NDT):
                        nc.tensor.matmul(ps[:], con_sb[:, C_WQK + 128 * dt:C_WQK + 128 * dt + 128],
                                         xt_sb[:, dt, sl],
                                         start=(dt == 0), stop=(dt == NDT - 1))
                    nc.scalar.activation(qT_sb[:, sl], ps[0:HD, :],
                                         mybir.ActivationFunctionType.Identity,
                                         bias=qb32[:, 0:1])
                    nc.vector.tensor_copy(kT_sb[:, sl], ps[HD:128, :])

                v_sb = vpool.tile([128, NQT, HD + 1], f16, tag="v", name=f"v_{b}")
                nc.vector.memset(v_sb[:, :, HD:HD + 1], 1.0)
                for st in range(NQT):
                    ps = ps512.tile([128, 512], f32, tag='ps', bufs=2)
                    for dt in range(NDT):
                        nc.tensor.matmul(ps[:, 0:HD],
                                         xt_sb[:, dt, 128 * st:128 * st + 128],
                                         con_sb[:, C_WV + HD * dt:C_WV + HD * dt + HD],
                                         start=(dt == 0), stop=(dt == NDT - 1))
                    nc.vector.tensor_copy(v_sb[:, st, 0:HD], ps[:, 0:HD])

                u_sb = kqpool.tile([128, S], f16, tag="u", name=f"u_{b}")
                for ch in range(NQC):
                    sl = slice(512 * ch, 512 * ch + 512)
                    ps = ps512.tile([128, 512], f32, tag='ps', bufs=2)
                    nc.tensor.matmul(ps[0:HD, :], con_sb[0:HD, C_POS:C_POS + FREQS],
                                     qT_sb[:, sl], start=True, stop=True)
                    nc.vector.tensor_mul(u_sb[0:64, sl], ps[0:HD, :], csq_sb[0:64, sl])
                    nc.vector.tensor_mul(u_sb[64:128, sl], ps[0:HD, :], csq_sb[64:128, sl])

                # ---- attention + row-parallel out projection partials ----
                for qc in range(NQC):
                    qsl = slice(512 * qc, 512 * qc + 512)
                    o_ps = pso.tile([HD + 1, 512], f32)
                    n_kt = 4 * qc + 4
                    for kt in range(n_kt):
                        s_ps = ps512.tile([128, 512], f32, tag='sps', bufs=2)
                        nc.tensor.matmul(s_ps[:], kT_sb[:, 128 * kt:128 * kt + 128],
                                         qT_sb[:, qsl], start=True, stop=False)
                        nc.tensor.matmul(s_ps[:], csk_sb[:, 128 * kt:128 * kt + 128],
                                         u_sb[:, qsl], start=False, stop=True)
                        m = kt - 4 * qc
                        if m > 0:
                            nc.vector.tensor_scalar_add(s_ps[:, 0:128 * m],
                                                        s_ps[:, 0:128 * m], -1e5)
                        if m >= 0:
                            msl = slice(128 * m, 128 * m + 128)
                            nc.vector.tensor_add(s_ps[:, msl], s_ps[:, msl], mask_sb[:])
                        p_sb = ptpool.tile([128, 512], f16, tag="pT")
                        nc.scalar.activation(p_sb[:], s_ps[:],
                                             mybir.ActivationFunctionType.Exp,
                                             scale=0.125, bias=nb4[:, 0:1])
                        nc.tensor.matmul(o_ps[:], v_sb[:, kt, :], p_sb[:],
                                         start=(kt == 0), stop=(kt == n_kt - 1))
                    recip = smpool.tile([1, 512], f32, tag="recip")
                    nc.vector.reciprocal(recip[:], o_ps[HD:HD + 1, :])
                    bcast = smpool.tile([HD, 512], f32, tag="bcast")
                    nc.gpsimd.partition_broadcast(bcast[:], recip[:])
                    o_sb = smpool.tile([HD, 512], f16, tag="osb")
                    nc.vector.tensor_mul(o_sb[:], o_ps[0:HD, :], bcast[:])
                    # out_w rows of this head: partial [128 outdim, 512 seq] per mt
                    for mt in range(NDT):
                        c0 = C_OW + 128 * mt
                        ps_o = ps512.tile([128, 512], f32, tag='pso2', bufs=2)
                        nc.tensor.matmul(ps_o[:], con_sb[0:HD, c0:c0 + 128],
                                         o_sb[:], start=True, stop=True)
                        po_sb = popool.tile([128, 512], f32, tag="po")
                        nc.vector.tensor_copy(po_sb[:], ps_o[:])
                        nc.sync.dma_start(
                            out=rs_in[2 * qc:2 * qc + 2, b,
                                      128 * mt:128 * mt + 128, :].rearrange(
                                          "c p s -> p c s"),
                            in_=po_sb[:].rearrange("p (c s) -> p c s", c=2))

            # AllToAll the per-head partials so core c holds every head's
            # contribution to output sequence slice c, then sum locally.
            # (ReduceScatter returns garbage on this runtime; AllToAll is the
            # proven path.)
            tc.strict_bb_all_engine_barrier()
            nc.gpsimd.collective_compute(
                "AllToAll", mybir.AluOpType.bypass,
                replica_groups=groups,
                ins=[rs_in[:]], outs=[rs_swap[:]])
            tc.strict_bb_all_engine_barrier()

            # ---- sum over heads + bias add + fp16 convert of this slice ----
            for b in range(B):
                for mt in range(NDT):
                    fo_sb = fopool.tile([128, N_CORES, QS], f32, tag="fo")
                    nc.sync.dma_start(
                        out=fo_sb[:],
                        in_=rs_swap[:, b, 128 * mt:128 * mt + 128, :].rearrange(
                            "g p s -> p g s"))
                    for g in (1, 3, 5, 7):
                        nc.vector.tensor_add(fo_sb[:, g - 1, :], fo_sb[:, g - 1, :],
                                             fo_sb[:, g, :])
                    for g in (2, 6):
                        nc.vector.tensor_add(fo_sb[:, g - 2, :], fo_sb[:, g - 2, :],
                                             fo_sb[:, g, :])
                    nc.vector.tensor_add(fo_sb[:, 0, :], fo_sb[:, 0, :],
                                         fo_sb[:, 4, :])
                    o16 = fopool.tile([128, QS], f16, tag="o16")
                    nc.scalar.activation(o16[:], fo_sb[:, 0, :],
                                         mybir.ActivationFunctionType.Identity,
                                         bias=ob32[:, mt:mt + 1])
                    nc.sync.dma_start(out=out[b, 128 * mt:128 * mt + 128, :], in_=o16[:])

    nc.finalize()
    return nc


def _get_nc():
    if "nc" not in _CACHE:
        _CACHE["nc"] = _build()
    return _CACHE["nc"]


def kernel(x, qkv, q_bias, positional, out_w, out_b, _want_results=False, _trace=False):
    from concourse.bass_utils import run_bass_kernel_spmd

    x = np.asarray(x, dtype=np.float32)
    qkv = np.asarray(qkv, dtype=np.float32)
    q_bias = np.asarray(q_bias, dtype=np.float32)
    positional = np.asarray(positional, dtype=np.float32)
    out_w = np.asarray(out_w, dtype=np.float32)
    out_b = np.asarray(out_b, dtype=np.float32)

    nc = _get_nc()

    # xT fp16, sharded by sequence: core c gets columns [QS*c : QS*(c+1)]
    xT = np.ascontiguousarray(x.transpose(0, 2, 1)).astype(np.float16)  # [B, X, S]

    in_maps = []
    for c in range(N_CORES):
        con = np.zeros((128, CC), dtype=np.float16)
        wqk = np.concatenate([qkv[:, 0, c, :], qkv[:, 1, c, :]], axis=1)  # [512, 128]
        con[:, C_WQK:C_WQK + 512] = wqk.reshape(NDT, 128, 128).transpose(1, 0, 2).reshape(128, 512)
        wv = qkv[:, 2, c, :]  # [512, 64]
        con[:, C_WV:C_WV + 256] = wv.reshape(NDT, 128, HD).transpose(1, 0, 2).reshape(128, 256)
        con[0:HD, C_POS:C_POS + FREQS] = positional[:, c, :].T  # [h, f]
        con[0:HD, C_QB] = q_bias[c]
        con[0:HD, C_OW:C_OW + X] = out_w[HD * c:HD * c + HD, :]  # [64, 512]
        con[:, C_OB:C_OB + NDT] = out_b.reshape(NDT, 128).T
        in_maps.append({
            "agin": np.concatenate([
                np.ascontiguousarray(xT[:, :, QS * c:QS * c + QS]).ravel(),
                con.ravel()]),
        })

    res = run_bass_kernel_spmd(nc, in_maps, core_ids=list(range(N_CORES)),
                               trace=_trace)
    outT = np.empty((B, X, S), dtype=np.float32)
    for c in range(N_CORES):
        outT[:, :, QS * c:QS * c + QS] = res.results[c]["out"].astype(np.float32)
    out = np.ascontiguousarray(outT.transpose(0, 2, 1))
    if _want_results:
        return out, res
    return out
